# revision 1
# baseline (speedup 1.0000x reference)
"""Epipolar correlation layer on 8 Trainium2 NeuronCores.

Sharding: data-parallel over (batch b, pixel half h) -> 8 shards.

Host precomputes sampling geometry (pair-gather indices + bilinear
weights), replicating the reference fp32-exactly. Device does the heavy
sampling: per (offset, superchunk), SWDGE dma_gather of row-pair 512B
bf16 pixel columns (transposed to channel-major), elementwise multiply
with imgL (DVE), and channel-sum dot products (PE ones-matmul). Device
outputs raw dot rows d[o, sc, s, r*SC+px]; the cheap weighted bilinear
combine runs on host in fp32.
"""
import numpy as np
import ml_dtypes

import concourse.bass as bass
import concourse.bacc as bacc
import concourse.mybir as mybir
from concourse import bass_utils
from concourse.library_config import mlp

B, C, H, W = 4, 96, 96, 320
HW = H * W
HWH = HW // 2              # 15360 pixels per core
MAXD = list(range(-4, 5))
MIND = list(range(-4, 5))
O = 81
ZERO_IDX = np.int32(HW)

SC = 3072                  # superchunk pixels
NSC = HWH // SC            # 5
NI = 2 * SC                # pair indices per gather call (r-major)
NI16 = NI // 16
NROW = HW + 128            # imgR rows incl. zero pad
NSEC = 4                   # PE/copy sections per unit: (s, half)
SECW = NI // 2             # 3072 columns per section

f32 = mybir.dt.float32
bf16 = mybir.dt.bfloat16
i16 = mybir.dt.int16

UNITS = [(sc, o) for sc in range(NSC) for o in range(O)]   # sc-outer
NU = len(UNITS)            # 405

_CACHE = {}


# ---------------------------------------------------------------- geometry
def _part1_jax(R, T, initial_flow):
    import jax
    import jax.numpy as jnp

    cpu = jax.devices("cpu")[0]

    def f(R, T, initial_flow):
        K = np.zeros((3, 3), np.float64)
        K[0, 0] = 0.89115971 * W
        K[0, 2] = 0.5 * W
        K[1, 1] = 1.18821287 * H
        K[1, 2] = 0.5 * H
        K[2, 2] = 1.0
        Kn = K.astype(np.float32)
        Ki = np.linalg.inv(K).astype(np.float32)
        jj, ii = np.meshgrid(np.arange(W), np.arange(H))
        pix_h = np.stack([jj, ii, np.ones_like(jj)], -1).reshape(-1, 3).astype(np.float32)
        pixel_dir = jnp.asarray(pix_h @ Ki.T)
        pixel_loc = jnp.asarray(np.stack([jj, ii], -1).astype(np.float32))
        Kj = jnp.asarray(Kn)
        KR = jnp.einsum('ij,bjk->bik', Kj, R)
        first_part = jnp.einsum('bij,nj->bni', KR, pixel_dir)
        second_part = jnp.einsum('ij,bjk->bik', Kj, T)[:, :, 0][:, None, :]

        def safe(d):
            return jnp.where(jnp.abs(d) < 1e-6, 1e-6, d)

        end_point = first_part[..., :2] / safe(first_part[..., 2:3])
        space_point = first_part * 10.0 + second_part
        project_point = space_point[..., :2] / safe(space_point[..., 2:3])
        diff = project_point - end_point
        para = diff / jnp.maximum(jnp.linalg.norm(diff, axis=-1, keepdims=True), 1e-12)
        perp = jnp.stack([-para[..., 1], para[..., 0]], axis=-1)
        para_r = para.reshape(B, H, W, 2)
        perp_r = perp.reshape(B, H, W, 2)
        end_r = end_point.reshape(B, H, W, 2)
        flow_point = pixel_loc[None] + jnp.transpose(initial_flow, (0, 2, 3, 1))
        nearest_k = jnp.sum((flow_point - end_r) * para_r, axis=3, keepdims=True)
        initial_loc = end_r + nearest_k * para_r
        epipolar_flow = jnp.transpose(initial_loc - pixel_loc[None], (0, 3, 1, 2))
        para_out = jnp.transpose(para_r, (0, 3, 1, 2))
        return initial_loc, para_r, perp_r, epipolar_flow, para_out

    with jax.default_device(cpu):
        args = [jax.device_put(np.asarray(x), cpu) for x in (R, T, initial_flow)]
        out = jax.jit(f, backend="cpu")(*args)
    return [np.asarray(x) for x in out]


def geometry(R, T, initial_flow):
    initial_loc, para, perp, epipolar_flow, para_out = _part1_jax(R, T, initial_flow)
    initial_loc = initial_loc.reshape(B, HW, 2)
    para = para.reshape(B, HW, 2)
    perp = perp.reshape(B, HW, 2)
    offsets = np.array([[p, q] for p in MAXD for q in MIND], np.float32)
    idx = np.empty((B, O, 2, HW), np.int32)
    wt = np.empty((B, O, 2, 2, HW), np.float32)
    Wn, Hn = np.float32(W), np.float32(H)
    one, two, half = np.float32(1.0), np.float32(2.0), np.float32(0.5)
    for o in range(O):
        para_i, perp_i = offsets[o, 0], offsets[o, 1]
        g = initial_loc + para_i * para + perp_i + perp
        gxn = two * g[..., 0] / (Wn - one) - one
        gyn = two * g[..., 1] / (Hn - one) - one
        gx = ((gxn + one) * Wn - one) * half
        gy = ((gyn + one) * Hn - one) * half
        x0 = np.floor(gx)
        y0 = np.floor(gy)
        wx = gx - x0
        wy = gy - y0
        in_x = (x0 >= 0) & (x0 <= W - 2)
        left = x0 == -1
        right = x0 == W - 1
        ws0 = np.where(in_x, one - wx, np.where(left, wx, 0.0)).astype(np.float32)
        ws1 = np.where(in_x, wx, np.where(right, one - wx, 0.0)).astype(np.float32)
        x_base = np.clip(x0, 0, W - 2).astype(np.int32)
        for r in range(2):
            yr = y0 + r
            vy = (yr >= 0) & (yr <= H - 1)
            wyr = (one - wy) if r == 0 else wy
            wrow = np.where(vy, wyr, 0.0).astype(np.float32)
            yc = np.clip(yr, 0, H - 1).astype(np.int32)
            row_idx = yc * W + x_base
            dead = (~vy) | ((ws0 == 0) & (ws1 == 0))
            idx[:, o, r, :] = np.where(dead, ZERO_IDX, row_idx)
            wt[:, o, r, 0, :] = wrow * ws0
            wt[:, o, r, 1, :] = wrow * ws1
    wt /= np.float32(C)
    return epipolar_flow, para_out, idx, wt


# ---------------------------------------------------------------- device
def build_program():
    nc = bacc.Bacc("TRN2", debug=False)
    imgr_d = nc.dram_tensor("imgr", [NROW, 128], bf16, kind="ExternalInput")
    imgl2_d = nc.dram_tensor("imgl2", [NSC, C, NI], bf16, kind="ExternalInput")
    idx_d = nc.dram_tensor("idx", [NU, 128, NI16], i16, kind="ExternalInput")
    d_out = nc.dram_tensor("dvals", [NU, 2, NI], f32, kind="ExternalOutput")

    imgr_pairs = bass.AP(imgr_d[:].tensor, 0, [[128, NROW - 1], [1, 256]])

    G = [nc.alloc_sbuf_tensor(f"g{i}", [128, 2, NI], bf16) for i in range(2)]
    l2_s = nc.alloc_sbuf_tensor("l2", [C, NI], bf16)
    idx_s = [nc.alloc_sbuf_tensor(f"ix{i}", [128, NI16], i16) for i in range(2)]
    dst_s = [nc.alloc_sbuf_tensor(f"d{i}", [33, NI], f32) for i in range(2)]
    ones_s = nc.alloc_sbuf_tensor("ones", [C, 1], bf16)
    psum_t = nc.alloc_psum_tensor("ps", [1, SECW], f32)

    s_idx = nc.alloc_semaphore("s_idx")      # 16 per idx load
    s_l2 = nc.alloc_semaphore("s_l2")        # 16 per l2 load
    s_g = nc.alloc_semaphore("s_g")          # 16 per gather
    s_mul = nc.alloc_semaphore("s_mul")      # 1 per unit
    s_pesec = nc.alloc_semaphore("s_pesec")  # 1 per section
    s_cp = nc.alloc_semaphore("s_cp")        # 1 per section copy
    s_out = nc.alloc_semaphore("s_out")      # 16 per out dma
    s_init = nc.alloc_semaphore("s_init")

    with nc.Block() as blk:

        @blk.vector
        def _(v):
            v.memset(ones_s[:], 1.0)
            v.engine_nop().then_inc(s_init, 1)
            for n1, (sc, o) in enumerate(UNITS):
                n = n1 + 1
                gbuf = G[n1 % 2]
                v.wait_ge(s_g, 16 * n)
                if o == 0:
                    v.wait_ge(s_l2, 16 * (sc + 1))
                ins = None
                for s in range(2):
                    ins = v.tensor_mul(gbuf[0:C, s, :], gbuf[0:C, s, :], l2_s[:, :])
                ins.then_inc(s_mul, 1)

        @blk.gpsimd
        def _(g):
            g.load_library(mlp)
            for n1, (sc, o) in enumerate(UNITS):
                n = n1 + 1
                gbuf = G[n1 % 2]
                g.wait_ge(s_idx, 16 * n)
                if n > 2:
                    g.wait_ge(s_pesec, NSEC * (n - 2))
                g.dma_gather(
                    gbuf[:], imgr_pairs, idx_s[n1 % 2][:],
                    NI, NI, 256, elem_step=128, transpose=True,
                    single_packet=False,
                ).then_inc(s_g, 16)

        @blk.tensor
        def _(t):
            t.wait_ge(s_init, 1)
            for n1, (sc, o) in enumerate(UNITS):
                n = n1 + 1
                gbuf = G[n1 % 2]
                t.wait_ge(s_mul, n)
                for si in range(NSEC):
                    s, hf = si // 2, si % 2
                    m = NSEC * n1 + si + 1
                    if m > 1:
                        t.wait_ge(s_cp, m - 1)
                    ins = None
                    for c in range(SECW // 512):
                        col = hf * SECW + c * 512
                        ins = t.matmul(
                            psum_t[:, c * 512:(c + 1) * 512],
                            ones_s[:],
                            gbuf[0:C, s, col:col + 512],
                            start=True, stop=True,
                        )
                    ins.then_inc(s_pesec, 1)

        @blk.scalar
        def _(se):
            for n1, (sc, o) in enumerate(UNITS):
                n = n1 + 1
                dbuf = dst_s[n1 % 2]
                for si in range(NSEC):
                    s, hf = si // 2, si % 2
                    m = NSEC * n1 + si + 1
                    se.wait_ge(s_pesec, m)
                    if si == 0 and n > 2:
                        se.wait_ge(s_out, 32 * (n - 2))
                    row = 32 * s
                    se.copy(
                        dbuf[row:row + 1, hf * SECW:(hf + 1) * SECW], psum_t[:]
                    ).then_inc(s_cp, 1)

        @blk.sync
        def _(sy):
            sy.dma_start(idx_s[0][:], idx_d[0]).then_inc(s_idx, 16)
            sy.dma_start(l2_s[:], imgl2_d[0]).then_inc(s_l2, 16)
            for n1, (sc, o) in enumerate(UNITS):
                n = n1 + 1
                # prefetch idx for unit n1+1 into the other buffer
                if n1 + 1 < NU:
                    if n1 >= 1:
                        sy.wait_ge(s_g, 16 * n1)
                    sy.dma_start(idx_s[(n1 + 1) % 2][:], idx_d[n1 + 1]).then_inc(s_idx, 16)
                # l2 slab for next sc (single-buffered; wait muls of this sc done)
                if o == O - 1 and sc + 1 < NSC:
                    sy.wait_ge(s_mul, n)
                    sy.dma_start(l2_s[:], imgl2_d[sc + 1]).then_inc(s_l2, 16)
                # out dma (one per s-plane; planes at partitions 0 and 32)
                sy.wait_ge(s_cp, NSEC * n)
                dbuf = dst_s[n1 % 2]
                sy.dma_start(d_out[n1, 0:1, :], dbuf[0:1, :]).then_inc(s_out, 16)
                sy.dma_start(d_out[n1, 1:2, :], dbuf[32:33, :]).then_inc(s_out, 16)
            sy.wait_ge(s_out, 32 * NU)

    nc.compile()
    nc.finalize()
    return nc


# ---------------------------------------------------------------- host glue
def prep_core_inputs(b, h, imgLb, imgRb, idx_b):
    imgr = np.zeros((NROW, 128), ml_dtypes.bfloat16)
    imgr[:HW, :C] = imgRb.reshape(C, HW).T.astype(ml_dtypes.bfloat16)

    p0 = h * HWH
    L = imgLb.reshape(C, HW)[:, p0:p0 + HWH].astype(ml_dtypes.bfloat16)
    imgl2 = np.empty((NSC, C, NI), ml_dtypes.bfloat16)
    for sc in range(NSC):
        sl = L[:, sc * SC:(sc + 1) * SC]
        imgl2[sc, :, :SC] = sl
        imgl2[sc, :, SC:] = sl

    idxh = idx_b[:, :, p0:p0 + HWH]            # (O, 2, HWH)
    # gather list position t = r*SC + px, per unit (sc, o)
    lists = np.empty((NU, NI), np.int16)
    for n1, (sc, o) in enumerate(UNITS):
        lists[n1, :SC] = idxh[o, 0, sc * SC:(sc + 1) * SC]
        lists[n1, SC:] = idxh[o, 1, sc * SC:(sc + 1) * SC]
    # wrap: position t -> partition t%16, slot t//16; replicate to 128 parts
    idx_w = lists.reshape(NU, NI16, 16).transpose(0, 2, 1)    # (NU, 16, NI16)
    idx_full = np.ascontiguousarray(np.tile(idx_w, (1, 8, 1)))
    return {"imgr": imgr, "imgl2": imgl2, "idx": idx_full}


def kernel(imgL, imgR, R, T, initial_flow):
    imgL = np.asarray(imgL)
    imgR = np.asarray(imgR)
    R = np.asarray(R)
    T = np.asarray(T)
    initial_flow = np.asarray(initial_flow)

    epipolar_flow, para_out, idx, wt = geometry(R, T, initial_flow)

    if "nc" not in _CACHE:
        _CACHE["nc"] = build_program()
    nc = _CACHE["nc"]

    in_maps = []
    for core in range(8):
        b, h = core // 2, core % 2
        in_maps.append(prep_core_inputs(b, h, imgL[b], imgR[b], idx[b]))

    res = bass_utils.run_bass_kernel_spmd(nc, in_maps, core_ids=list(range(8)),
                                          trace=False)

    out = np.empty((B, 4 + O, H, W), np.float32)
    out[:, 0:2] = epipolar_flow
    out[:, 2:4] = para_out
    corr = out[:, 4:].reshape(B, O, HW)
    for core in range(8):
        b, h = core // 2, core % 2
        p0 = h * HWH
        d = res.results[core]["dvals"].reshape(NSC, O, 2, 2, SC)  # (sc,o,s,r,SC)
        # -> (O, r, s, HWH)
        da = d.transpose(1, 3, 2, 0, 4).reshape(O, 2, 2, HWH)
        wth = wt[b, :, :, :, p0:p0 + HWH]                         # (O, r, s, HWH)
        corr[b, :, p0:p0 + HWH] = np.einsum('orsp,orsp->op', wth, da)
    return out



# revision 4
# speedup vs baseline: 57.8069x; 57.8069x over previous
"""Epipolar correlation layer on 8 Trainium2 NeuronCores.

Tile-GEMM architecture. The 81 correlation offsets for a tile of 64
output pixels sample a heavily-overlapping set of imgR pixels; instead
of gathering 4 neighbor pixel-vectors per (pixel, offset) like a direct
grid_sample (2.5M 512B descriptors/core), each tile gathers the
DEDUPLICATED union of needed imgR pixels once (~11x fewer bytes), and
one PE matmul computes all dot products D[p, q] = sum_c imgL[c,p] *
imgR[c,q] for the tile. The cheap per-sample bilinear combine
(4 weighted D values per sample) runs on host in fp32, as in the
baseline.

Device per unit (= tile chunk, static shapes): load idx + imgL slab,
SWDGE-gather up to 1024 imgR pixel columns (256 B each, round-robin
over 4 SWDGE queues), matmul [96x64]^T x [96x1024] -> PSUM, copy to
bf16, DMA out. A gpsimd If/Else on a per-unit meta word selects which
of the core's two assigned batches the gather reads (gather indices are
int16 so one batch's 30720 pixels fit, two batches' do not).

Work assignment: only ~48% of tiles have any in-bounds sample (random
R/T make many epipolar lines leave the image); inactive tiles are
skipped entirely. Active units are balanced across cores, each core
serving at most 2 batches.
"""
import numpy as np
import ml_dtypes

import concourse.bass as bass
import concourse.bacc as bacc
import concourse.mybir as mybir
from concourse import bass_utils
from concourse.library_config import mlp

B, C, H, W = 4, 96, 96, 320
HW = H * W
MAXD = list(range(-4, 5))
MIND = list(range(-4, 5))
O = 81
ZERO_IDX = np.int32(HW)

TS = 64                    # output pixels per tile
NQ = 1024                  # q slots per unit
NQ16 = NQ // 16
NT = HW // TS              # tiles per batch image
NCORE = 8
NQUEUE = 4
BUFS = 8

f32 = mybir.dt.float32
bf16 = mybir.dt.bfloat16
i16 = mybir.dt.int16
i32 = mybir.dt.int32

_CACHE = {}


# ---------------------------------------------------------------- geometry
def _part1_jax(R, T, initial_flow):
    import jax
    import jax.numpy as jnp

    cpu = jax.devices("cpu")[0]

    def f(R, T, initial_flow):
        K = np.zeros((3, 3), np.float64)
        K[0, 0] = 0.89115971 * W
        K[0, 2] = 0.5 * W
        K[1, 1] = 1.18821287 * H
        K[1, 2] = 0.5 * H
        K[2, 2] = 1.0
        Kn = K.astype(np.float32)
        Ki = np.linalg.inv(K).astype(np.float32)
        jj, ii = np.meshgrid(np.arange(W), np.arange(H))
        pix_h = np.stack([jj, ii, np.ones_like(jj)], -1).reshape(-1, 3).astype(np.float32)
        pixel_dir = jnp.asarray(pix_h @ Ki.T)
        pixel_loc = jnp.asarray(np.stack([jj, ii], -1).astype(np.float32))
        Kj = jnp.asarray(Kn)
        KR = jnp.einsum('ij,bjk->bik', Kj, R)
        first_part = jnp.einsum('bij,nj->bni', KR, pixel_dir)
        second_part = jnp.einsum('ij,bjk->bik', Kj, T)[:, :, 0][:, None, :]

        def safe(d):
            return jnp.where(jnp.abs(d) < 1e-6, 1e-6, d)

        end_point = first_part[..., :2] / safe(first_part[..., 2:3])
        space_point = first_part * 10.0 + second_part
        project_point = space_point[..., :2] / safe(space_point[..., 2:3])
        diff = project_point - end_point
        para = diff / jnp.maximum(jnp.linalg.norm(diff, axis=-1, keepdims=True), 1e-12)
        perp = jnp.stack([-para[..., 1], para[..., 0]], axis=-1)
        para_r = para.reshape(B, H, W, 2)
        perp_r = perp.reshape(B, H, W, 2)
        end_r = end_point.reshape(B, H, W, 2)
        flow_point = pixel_loc[None] + jnp.transpose(initial_flow, (0, 2, 3, 1))
        nearest_k = jnp.sum((flow_point - end_r) * para_r, axis=3, keepdims=True)
        initial_loc = end_r + nearest_k * para_r
        epipolar_flow = jnp.transpose(initial_loc - pixel_loc[None], (0, 3, 1, 2))
        para_out = jnp.transpose(para_r, (0, 3, 1, 2))
        return initial_loc, para_r, perp_r, epipolar_flow, para_out

    with jax.default_device(cpu):
        args = [jax.device_put(np.asarray(x), cpu) for x in (R, T, initial_flow)]
        out = jax.jit(f, backend="cpu")(*args)
    return [np.asarray(x) for x in out]


def geometry(R, T, initial_flow):
    initial_loc, para, perp, epipolar_flow, para_out = _part1_jax(R, T, initial_flow)
    initial_loc = initial_loc.reshape(B, HW, 2)
    para = para.reshape(B, HW, 2)
    perp = perp.reshape(B, HW, 2)
    offsets = np.array([[p, q] for p in MAXD for q in MIND], np.float32)
    idx = np.empty((B, O, 2, HW), np.int32)
    wt = np.empty((B, O, 2, 2, HW), np.float32)
    Wn, Hn = np.float32(W), np.float32(H)
    one, two, half = np.float32(1.0), np.float32(2.0), np.float32(0.5)
    for o in range(O):
        para_i, perp_i = offsets[o, 0], offsets[o, 1]
        g = initial_loc + para_i * para + perp_i + perp
        gxn = two * g[..., 0] / (Wn - one) - one
        gyn = two * g[..., 1] / (Hn - one) - one
        gx = ((gxn + one) * Wn - one) * half
        gy = ((gyn + one) * Hn - one) * half
        x0 = np.floor(gx)
        y0 = np.floor(gy)
        wx = gx - x0
        wy = gy - y0
        in_x = (x0 >= 0) & (x0 <= W - 2)
        left = x0 == -1
        right = x0 == W - 1
        ws0 = np.where(in_x, one - wx, np.where(left, wx, 0.0)).astype(np.float32)
        ws1 = np.where(in_x, wx, np.where(right, one - wx, 0.0)).astype(np.float32)
        x_base = np.clip(x0, 0, W - 2).astype(np.int32)
        for r in range(2):
            yr = y0 + r
            vy = (yr >= 0) & (yr <= H - 1)
            wyr = (one - wy) if r == 0 else wy
            wrow = np.where(vy, wyr, 0.0).astype(np.float32)
            yc = np.clip(yr, 0, H - 1).astype(np.int32)
            row_idx = yc * W + x_base
            dead = (~vy) | ((ws0 == 0) & (ws1 == 0))
            idx[:, o, r, :] = np.where(dead, ZERO_IDX, row_idx)
            wt[:, o, r, 0, :] = wrow * ws0
            wt[:, o, r, 1, :] = wrow * ws1
    wt /= np.float32(C)
    return epipolar_flow, para_out, idx, wt


# ---------------------------------------------------------------- packing
def pack(idx):
    """Build tile q-lists and unit descriptors from the row-index map."""
    idx_t = idx.reshape(B, O, 2, NT, TS)
    tiles = []     # (b, t, u_sorted_pixels)
    for b in range(B):
        rows_b = idx_t[b]                     # (O, 2, NT, TS)
        for t in range(NT):
            rows = rows_b[:, :, t, :]
            valid = rows != ZERO_IDX
            if not valid.any():
                continue
            rv = rows[valid].astype(np.int64)
            u = np.unique(np.concatenate([rv, rv + 1]))
            tiles.append((b, t, u))

    # units: chunks of <= NQ pixels, grouped by batch for core assignment
    units_by_batch = [[] for _ in range(B)]
    tile_units = []   # per tile: list of unit refs (filled with core/slot later)
    for ti, (b, t, u) in enumerate(tiles):
        nch = (len(u) + NQ - 1) // NQ
        refs = []
        for c in range(nch):
            chunk = u[c * NQ:(c + 1) * NQ]
            rec = {"b": b, "t": t, "pix": chunk, "tile": ti}
            units_by_batch[b].append(rec)
            refs.append(rec)
        tile_units.append(refs)

    # sequential fill: contiguous spans over batch-ordered stream,
    # <= 2 batches per core
    stream = []
    for b in range(B):
        stream.extend(units_by_batch[b])
    total = len(stream)
    cores = [[] for _ in range(NCORE)]
    pos = 0
    for ci in range(NCORE):
        remaining_cores = NCORE - ci
        quota = int(np.ceil((total - pos) / remaining_cores))
        batches_here = []
        while pos < total and len(cores[ci]) < quota:
            rec = stream[pos]
            if rec["b"] not in batches_here:
                if len(batches_here) == 2:
                    break
                batches_here.append(rec["b"])
            rec["core"] = ci
            rec["slot"] = len(cores[ci])
            cores[ci].append(rec)
            pos += 1
    assert pos == total, "unit assignment failed"
    return tiles, tile_units, cores


# ---------------------------------------------------------------- device
def build_program(nu):
    nc = bacc.Bacc("TRN2", debug=False, num_swdge_queues=NQUEUE)
    imgr_d = nc.dram_tensor("imgr", [2 * HW, 128], bf16, kind="ExternalInput")
    idx_d = nc.dram_tensor("idx", [nu, 128, NQ16], i16, kind="ExternalInput")
    meta_d = nc.dram_tensor("meta", [1, nu], i32, kind="ExternalInput")
    ls_d = nc.dram_tensor("ls", [nu, C, TS], bf16, kind="ExternalInput")
    d_out = nc.dram_tensor("dvals", [nu, TS, NQ], bf16, kind="ExternalOutput")

    src = [
        bass.AP(imgr_d[:].tensor, half * HW * 128, [[128, HW], [1, 128]])
        for half in range(2)
    ]

    Rq = [nc.alloc_sbuf_tensor(f"rq{i}", [128, 1, NQ], bf16) for i in range(BUFS)]
    idx_s = [nc.alloc_sbuf_tensor(f"ix{i}", [128, NQ16], i16) for i in range(BUFS)]
    ls_s = [nc.alloc_sbuf_tensor(f"l{i}", [C, TS], bf16) for i in range(BUFS)]
    dst_s = [nc.alloc_sbuf_tensor(f"d{i}", [TS, NQ], bf16) for i in range(BUFS)]
    meta_s = nc.alloc_sbuf_tensor("meta_s", [1, nu], i32)
    psum = [nc.alloc_psum_tensor(f"ps{i}", [TS, NQ], f32) for i in range(2)]

    s_meta = nc.alloc_semaphore("s_meta")
    s_idx = nc.alloc_semaphore("s_idx")
    s_ls = nc.alloc_semaphore("s_ls")
    s_g = [nc.alloc_semaphore(f"s_g{q}") for q in range(NQUEUE)]
    s_mm = nc.alloc_semaphore("s_mm")
    s_cp = nc.alloc_semaphore("s_cp")
    s_out = nc.alloc_semaphore("s_out")

    with nc.Block() as blk:

        @blk.gpsimd
        def _(g):
            g.load_library(mlp)
            g.wait_ge(s_meta, 16)
            with g.register("rb") as rb:
                for u in range(nu):
                    g.wait_ge(s_idx, 16 * (u + 1))
                    if u >= BUFS:
                        g.wait_ge(s_mm, u - BUFS + 1)
                    g.reg_load(rb, meta_s[0:1, u:u + 1])
                    with g.If_eq(rb, 0):
                        g.dma_gather(
                            Rq[u % BUFS][:], src[0], idx_s[u % BUFS][:],
                            NQ, NQ, 128, elem_step=128, transpose=True,
                            single_packet=False, queue_num=u % NQUEUE,
                        ).then_inc(s_g[u % NQUEUE], 16)
                    with g.Else():
                        g.dma_gather(
                            Rq[u % BUFS][:], src[1], idx_s[u % BUFS][:],
                            NQ, NQ, 128, elem_step=128, transpose=True,
                            single_packet=False, queue_num=u % NQUEUE,
                        ).then_inc(s_g[u % NQUEUE], 16)

        @blk.tensor
        def _(t):
            for u in range(nu):
                t.wait_ge(s_g[u % NQUEUE], 16 * (u // NQUEUE + 1))
                t.wait_ge(s_ls, 16 * (u + 1))
                if u >= 2:
                    t.wait_ge(s_cp, u - 1)
                ins = None
                for c in range(NQ // 512):
                    ins = t.matmul(
                        psum[u % 2][:, c * 512:(c + 1) * 512],
                        ls_s[u % BUFS][:],
                        Rq[u % BUFS][0:C, 0, c * 512:(c + 1) * 512],
                        start=True, stop=True,
                    )
                ins.then_inc(s_mm, 1)

        @blk.scalar
        def _(se):
            for u in range(nu):
                se.wait_ge(s_mm, u + 1)
                if u >= BUFS:
                    se.wait_ge(s_out, 16 * (u - BUFS + 1))
                se.copy(dst_s[u % BUFS][:], psum[u % 2][:]).then_inc(s_cp, 1)

        @blk.sync
        def _(sy):
            LA = 6

            def loads(u):
                if u >= BUFS:
                    uo = u - BUFS
                    sy.wait_ge(s_g[uo % NQUEUE], 16 * (uo // NQUEUE + 1))
                sy.dma_start(idx_s[u % BUFS][:], idx_d[u]).then_inc(s_idx, 16)
                if u >= BUFS:
                    sy.wait_ge(s_mm, u - BUFS + 1)
                sy.dma_start(ls_s[u % BUFS][:], ls_d[u]).then_inc(s_ls, 16)

            sy.dma_start(meta_s[:], meta_d[0]).then_inc(s_meta, 16)
            for u in range(min(LA, nu)):
                loads(u)
            for u in range(nu):
                if u + LA < nu:
                    loads(u + LA)
                sy.wait_ge(s_cp, u + 1)
                sy.dma_start(d_out[u], dst_s[u % BUFS][:]).then_inc(s_out, 16)
            sy.wait_ge(s_out, 16 * nu)

    nc.compile()
    nc.finalize()
    return nc


# ---------------------------------------------------------------- host glue
def prepare(imgL, imgR, R, T, initial_flow):
    epipolar_flow, para_out, idx, wt = geometry(R, T, initial_flow)
    tiles, tile_units, cores = pack(idx)
    nu = max(len(c) for c in cores)

    imgR_rows = np.zeros((B, HW, 128), ml_dtypes.bfloat16)
    imgR_rows[:, :, :C] = np.transpose(
        imgR.reshape(B, C, HW), (0, 2, 1)).astype(ml_dtypes.bfloat16)
    imgL_cm = imgL.reshape(B, C, HW).astype(ml_dtypes.bfloat16)

    in_maps = []
    for ci in range(NCORE):
        units = cores[ci]
        batches = []
        for rec in units:
            if rec["b"] not in batches:
                batches.append(rec["b"])
        while len(batches) < 2:
            batches.append(batches[0] if batches else 0)

        imgr2 = np.concatenate([imgR_rows[batches[0]], imgR_rows[batches[1]]], axis=0)
        idxmat = np.zeros((nu, NQ), np.int16)
        meta = np.zeros((1, nu), np.int32)
        ls = np.zeros((nu, C, TS), ml_dtypes.bfloat16)
        for rec in units:
            s = rec["slot"]
            npx = len(rec["pix"])
            idxmat[s, :npx] = rec["pix"].astype(np.int16)
            meta[0, s] = batches.index(rec["b"])
            ls[s] = imgL_cm[rec["b"]][:, rec["t"] * TS:(rec["t"] + 1) * TS]
        idx_w = idxmat.reshape(nu, NQ16, 16).transpose(0, 2, 1)
        idx_full = np.ascontiguousarray(np.tile(idx_w, (1, 8, 1)))
        in_maps.append({
            "imgr": np.ascontiguousarray(imgr2),
            "idx": idx_full,
            "meta": meta,
            "ls": np.ascontiguousarray(ls),
        })

    key = ("v2", nu)
    if key not in _CACHE:
        _CACHE[key] = build_program(nu)
    nc = _CACHE[key]
    ctx = (epipolar_flow, para_out, idx, wt, tiles, tile_units)
    return nc, in_maps, ctx


def combine(ctx, results):
    epipolar_flow, para_out, idx, wt, tiles, tile_units = ctx
    idx_t = idx.reshape(B, O, 2, NT, TS)
    wt_t = wt.reshape(B, O, 2, 2, NT, TS)
    dcores = [np.asarray(results[ci]["dvals"]) for ci in range(NCORE)]

    corr = np.zeros((B, O, HW), np.float32)
    ar = np.arange(TS)
    for ti, (b, t, u) in enumerate(tiles):
        refs = tile_units[ti]
        dtile = np.concatenate(
            [dcores[rec["core"]][rec["slot"]] for rec in refs], axis=1
        ).astype(np.float32)                      # (TS, nch*NQ)
        rows = idx_t[b, :, :, t, :]               # (O, 2, TS)
        valid = rows != ZERO_IDX
        rr = rows.clip(0, None).astype(np.int64)
        slot0 = np.searchsorted(u, rr)
        slot1 = np.searchsorted(u, rr + 1)
        hi = len(u) - 1
        pidx = np.broadcast_to(ar, rows.shape)
        v0 = dtile[pidx, slot0.clip(0, hi)]
        v1 = dtile[pidx, slot1.clip(0, hi)]
        w0 = wt_t[b, :, :, 0, t, :]
        w1 = wt_t[b, :, :, 1, t, :]
        contrib = np.where(valid, w0 * v0 + w1 * v1, 0.0)
        corr[b, :, t * TS:(t + 1) * TS] += contrib.sum(axis=1)

    out = np.empty((B, 4 + O, H, W), np.float32)
    out[:, 0:2] = epipolar_flow
    out[:, 2:4] = para_out
    out[:, 4:] = corr.reshape(B, O, H, W)
    return out


def kernel(imgL, imgR, R, T, initial_flow):
    imgL = np.asarray(imgL)
    imgR = np.asarray(imgR)
    R = np.asarray(R)
    T = np.asarray(T)
    initial_flow = np.asarray(initial_flow)

    nc, in_maps, ctx = prepare(imgL, imgR, R, T, initial_flow)
    res = bass_utils.run_bass_kernel_spmd(nc, in_maps, core_ids=list(range(NCORE)),
                                          trace=False)
    return combine(ctx, res.results)
